# revision 111
# baseline (speedup 1.0000x reference)
"""Multi-head attention (B=4, S=2048, D=1024, H=16) on 8 trn2 NeuronCores.

Sharding: batch x head-group (tensor parallel over heads). Core c handles
batch c//2 and heads (c%2)*8 .. (c%2)*8+7: it projects Q/K/V only for its
512 head dims (columns of Wq/Wk/Wv), runs attention for its 8 heads over
the full 2048-token sequence, and computes the PARTIAL output projection
y_g = O_g @ Wo[:, g-slice]^T (+ bias folded into group 0). The host adds
the two partials per batch during unshard - the row-sharded-Wo all-reduce
of standard tensor parallelism. No K/V projection duplication and no
cross-core traffic on device.

Device-side layout notes:
 - Activations stay transposed ([feature, token]); scores are computed as
   S^T[k, q] = K_h Q_h^T with head pairs stacked in partition halves
   (2 matmuls per 128-k tile, N=512 each), one ScalarE exp per [128,1024]
   PSUM tile.
 - AV is restructured for minimal PE row-streaming: the exp tile e[k, q]
   is the STATIONARY operand (lhsT, [128k x 128q] slices) and the moving
   operand is [V_h | ones] ([128k x 65]) so each matmul streams only 65
   rows -> out[q, 64+1] accumulates O[q, dk] AND the softmax denominator
   (col 64) over the 16 k-tiles. Per-partition reciprocal + tensor_scalar
   normalization (no cross-partition replicate needed), then a PE
   transpose (vs a DMA'd 128x128 identity) restores O^T[d, q] for the
   output projection.
 - AV runs TWO k-tiles behind the QK/exp stream (the scores double buffer
   caps ACT's lag at 2 tiles), so AV matmuls never stall on the exp that
   was just issued - this keeps every steady-state window at the ScalarE
   floor of 16 exp instructions.
 - Projections (K, Q, V, O) are emitted just-in-time inside the attention
   kt-loops to fill PE during ACT-bound stretches, split into half-chains
   (shared PSUM accumulation) so no fill lump exceeds what the 2-deep exp
   pipeline can absorb; V is projected in two head-halves (heads 0-3
   feed pairs 0-1, heads 4-7 feed pairs 2-3) to spread its cost. A few
   warm-up matmuls on a zeroed scratch tile hold the PE p-state ramp
   while the first DMAs land.
 - V bias is folded into the (partial) output-projection bias host-side;
   partial outputs are written back as bf16 (summed in f32 on the host).
 - PSUM budget: scores 2x[128,1024] + AV accumulators 2x[128,512] +
   projection/transpose 2x[128,512] = 8 banks exactly.
"""

import numpy as np

B, S, D, H = 4, 2048, 1024, 16
DK = D // H          # 64
HL = H // 2          # 8 local heads per core
DG = HL * DK         # 512 local head dims
CW = 512             # token chunk width
QC = S // CW         # 4 query chunks
KTN = S // 128       # 16 k tiles
HPN = HL // 2        # 4 local head pairs
SCALE = 1.0 / np.sqrt(DK)
N_CORES = 8

_CACHE = {}


def _build_program(reps=1):
    import concourse.bass as bass
    import concourse.mybir as mybir
    from concourse import bacc
    from concourse.tile import TileContext

    f32 = mybir.dt.float32
    bf16 = mybir.dt.bfloat16
    AF = mybir.ActivationFunctionType

    nc = bacc.Bacc("TRN2", target_bir_lowering=False)

    xqT = nc.declare_dram_parameter("xqT", [D, S], bf16, isOutput=False)
    xkT = nc.declare_dram_parameter("xkT", [D, S], bf16, isOutput=False)
    xvT = nc.declare_dram_parameter("xvT", [D, S], bf16, isOutput=False)
    # weights pre-blocked host-side to partition-major so sliced DMA loads
    # move 2KB+ contiguous runs (256B runs pay a 2x descriptor latency)
    wq_in = nc.declare_dram_parameter("wq_in", [128, 4, 8, 128], bf16,
                                      isOutput=False)
    wk_in = nc.declare_dram_parameter("wk_in", [128, 4, 8, 128], bf16,
                                      isOutput=False)
    wv_in = nc.declare_dram_parameter("wv_in", [128, 2, 8, 256], bf16,
                                      isOutput=False)
    woT = nc.declare_dram_parameter("woT", [DG, D], bf16, isOutput=False)
    bq_in = nc.declare_dram_parameter("bq_in", [128, 4], f32, isOutput=False)
    bk_in = nc.declare_dram_parameter("bk_in", [128, 4], f32, isOutput=False)
    bo_in = nc.declare_dram_parameter("bo_in", [128, 8], f32, isOutput=False)
    id_in = nc.declare_dram_parameter("id_in", [128, 128], bf16,
                                      isOutput=False)
    yT = nc.declare_dram_parameter("yT", [D, S], bf16, isOutput=True)

    xq_r = xqT[:].rearrange("(a p) t -> p a t", p=128)
    xk_r = xkT[:].rearrange("(a p) t -> p a t", p=128)
    xv_r = xvT[:].rearrange("(a p) t -> p a t", p=128)
    wq_r = wq_in[:]
    wk_r = wk_in[:]
    wv_r = wv_in[:]
    wo_r = woT[:].rearrange("(a p) d -> p a d", p=128)

    with TileContext(nc) as tc:
        for _rep in range(reps):
            _emit_body(nc, tc, bass, f32, bf16, AF,
                       xq_r, xk_r, xv_r, wq_r, wk_r, wv_r, wo_r,
                       bq_in, bk_in, bo_in, id_in, yT)
    nc.compile()
    return nc


def _emit_body(nc, tc, bass, f32, bf16, AF,
               xq_r, xk_r, xv_r, wq_r, wk_r, wv_r, wo_r,
               bq_in, bk_in, bo_in, id_in, yT):
    def mm(out, lhsT, rhs, start, stop):
        nc.tensor.matmul(out, lhsT=lhsT, rhs=rhs, start=start, stop=stop)

    with (
        tc.tile_pool(name="const", bufs=1) as const_pool,
        tc.tile_pool(name="kt_res", bufs=1) as kt_pool,
        tc.tile_pool(name="qt_res", bufs=1) as qt_pool,
        tc.tile_pool(name="vp_res", bufs=1) as vp_pool,
        tc.tile_pool(name="w_res", bufs=1) as w_pool,
        tc.tile_pool(name="xk_p", bufs=4) as xk_pool,
        tc.tile_pool(name="xv_p", bufs=4) as xv_pool,
        tc.tile_pool(name="xq_p", bufs=2) as xq_pool,
        tc.tile_pool(name="exp_p", bufs=6) as exp_pool,
        tc.tile_pool(name="on_p", bufs=3) as on_pool,
        tc.tile_pool(name="rec_p", bufs=3) as rec_pool,
        tc.tile_pool(name="ot_res", bufs=3) as ot_pool,
        tc.tile_pool(name="y_p", bufs=6) as y_pool,
        tc.tile_pool(name="ps_proj", bufs=2, space="PSUM") as ps_proj,
        tc.tile_pool(name="ps_s", bufs=2, space="PSUM") as ps_s,
        tc.tile_pool(name="ps_o", bufs=2, space="PSUM") as ps_o,
    ):
        bq_sb = const_pool.tile([128, 4], f32, tag="bq")
        bk_sb = const_pool.tile([128, 4], f32, tag="bk")
        bo_sb = const_pool.tile([128, 8], f32, tag="bo")
        id_sb = const_pool.tile([128, 128], bf16, tag="ident")
        nc.gpsimd.dma_start(out=bk_sb, in_=bk_in[:])
        nc.gpsimd.dma_start(out=bq_sb, in_=bq_in[:])
        nc.gpsimd.dma_start(out=bo_sb, in_=bo_in[:])
        nc.gpsimd.dma_start(out=id_sb, in_=id_in[:])

        KT_sb = kt_pool.tile([128, 4, S], bf16, tag="KT")   # [p, dj, t]
        QT_sb = qt_pool.tile([128, 4, S], bf16, tag="QT")   # [p, dj, t]
        Vp_sb = vp_pool.tile([128, KTN, HL * (DK + 1)], bf16, tag="Vp")
        Vp4 = Vp_sb.rearrange("p i (hh c) -> p i hh c", c=DK + 1)

        # scratch for PE p-state warm-up (zeros; results unused) —
        # memset first so the first warm matmul starts ASAP
        wsc = const_pool.tile([128, 384], bf16, tag="wsc")
        nc.vector.memset(wsc, 0.0)
        nc.vector.memset(Vp4[:, :, :, DK], 1.0)

        warm_ctr = [0]

        def warm(n):
            for _ in range(n):
                ps = ps_s.tile([128, 1024], f32, tag="pss",
                               name=f"warm_{warm_ctr[0]}")
                warm_ctr[0] += 1
                mm(ps[:, 0:256], wsc[:, 0:128], wsc[:, 128:384], True, True)

        wk_sb = w_pool.tile([128, 4, 8, 128], bf16, tag="wk")
        wq_sb = w_pool.tile([128, 4, 8, 128], bf16, tag="wq")
        wv_sb = w_pool.tile([128, 2, 8, 256], bf16, tag="wv")
        wo_sb = w_pool.tile([128, 4, D], bf16, tag="wo")

        xk_t, xv_t, xq_t = {}, {}, {}

        def load_xr(kind, tc_i, lo, hi):
            pool, cache, src = {
                "k": (xk_pool, xk_t, xk_r),
                "v": (xv_pool, xv_t, xv_r),
                "q": (xq_pool, xq_t, xq_r),
            }[kind]
            if tc_i not in cache:
                cache[tc_i] = pool.tile([128, 8, CW], bf16, tag=f"x{kind}",
                                        name=f"x{kind}_{tc_i}")
            base = tc_i * CW
            nc.sync.dma_start(out=cache[tc_i][:, :, lo:hi],
                              in_=src[:, :, base + lo:base + hi])

        def load_x(kind, tc_i):
            load_xr(kind, tc_i, 0, CW)

        def k_proj(dj, tci):
            ps = ps_proj.tile([128, 512], f32, tag="pp",
                              name=f"pk_{dj}_{tci}")
            for kj in range(8):
                mm(ps, wk_sb[:, dj, kj, :],
                   xk_t[tci][:, kj, :], kj == 0, kj == 7)
            nc.vector.tensor_scalar_add(
                out=KT_sb[:, dj, tci * CW:(tci + 1) * CW],
                in0=ps, scalar1=bk_sb[:, dj:dj + 1])

        def kq_proj_h(which, dj, tci, half):
            # half-width (256-token) chain: lets the prefix start on the
            # first half-chunk DMA instead of waiting for a full chunk
            w_sb, x_t, b_sb, out_sb = (
                (wk_sb, xk_t, bk_sb, KT_sb) if which == "k"
                else (wq_sb, xq_t, bq_sb, QT_sb))
            ps = ps_proj.tile([128, 512], f32, tag="pp",
                              name=f"p{which}h_{dj}_{tci}_{half}")
            lo = half * 256
            for kj in range(8):
                mm(ps[:, 0:256], w_sb[:, dj, kj, :],
                   x_t[tci][:, kj, lo:lo + 256], kj == 0, kj == 7)
            nc.vector.tensor_scalar_add(
                out=out_sb[:, dj, tci * CW + lo:tci * CW + lo + 256],
                in0=ps[:, 0:256], scalar1=b_sb[:, dj:dj + 1])

        def q_proj(qc, dj):
            for t in q_proj_split(qc, dj):
                t()

        def q_proj_split(qc, dj):
            # two half-chains sharing one PSUM accumulation, so the fill
            # can be spread across the window without a >1.4us PE lump
            box = {}

            def a():
                box["ps"] = ps_proj.tile([128, 512], f32, tag="pp",
                                         name=f"pq_{qc}_{dj}")
                for kj in range(4):
                    mm(box["ps"], wq_sb[:, dj, kj, :],
                       xq_t[qc][:, kj, :], kj == 0, False)

            def b():
                ps = box["ps"]
                for kj in range(4, 8):
                    mm(ps, wq_sb[:, dj, kj, :],
                       xq_t[qc][:, kj, :], False, kj == 7)
                nc.vector.tensor_scalar_add(
                    out=QT_sb[:, dj, qc * CW:(qc + 1) * CW],
                    in0=ps, scalar1=bq_sb[:, dj:dj + 1])

            return a, b

        def v_proj(ti, half):
            # half 0: heads 0-3 (used by hp0/hp1), half 1: heads 4-7
            tci, ts = ti // 4, ti % 4
            ps = ps_proj.tile([128, 512], f32, tag="pp",
                              name=f"pv_{ti}_{half}")
            for kj in range(8):
                mm(ps[:, 0:256], xv_t[tci][:, kj, ts * 128:(ts + 1) * 128],
                   wv_sb[:, half, kj, :], kj == 0, kj == 7)
            nc.vector.tensor_copy(
                out=Vp4[:, ti, half * 4:(half + 1) * 4, 0:DK],
                in_=ps[:, 0:256].rearrange("p (hh c) -> p hh c", c=DK))

        OT_tiles = {}

        def o_proj_split(qc, dj, pool=None, on_act=False):
            box = {}

            def a():
                p = pool or ps_proj
                t = p.tile([128, 512] if p is ps_proj else [128, 1024],
                           f32, tag="pp" if p is ps_proj else "pss",
                           name=f"py_{qc}_{dj}")
                box["ps"] = t[:, 0:512]
                for kj in range(3):
                    mm(box["ps"], wo_sb[:, kj, dj * 128:(dj + 1) * 128],
                       OT_tiles[qc][:, kj, :], kj == 0, False)

            def b():
                ps_y = box["ps"]
                mm(ps_y, wo_sb[:, 3, dj * 128:(dj + 1) * 128],
                   OT_tiles[qc][:, 3, :], False, True)
                yt = y_pool.tile([128, 512], bf16, tag="yt",
                                 name=f"yt_{qc}_{dj}")
                if on_act:
                    # tail-only: ACT is idle after the last exp; keep the
                    # serial DVE queue off the finish-line critical path
                    nc.scalar.activation(out=yt, in_=ps_y, func=AF.Identity,
                                         bias=bo_sb[:, dj:dj + 1], scale=1.0)
                else:
                    nc.vector.tensor_scalar_add(
                        out=yt, in0=ps_y, scalar1=bo_sb[:, dj:dj + 1])
                nc.sync.dma_start(
                    out=yT[dj * 128:(dj + 1) * 128, qc * CW:(qc + 1) * CW],
                    in_=yt)

            return a, b

        def o_proj(qc, dj, pool=None):
            for t in o_proj_split(qc, dj, pool):
                t()

        # deferred per-head-pair transposes: Onorm[q, d] -> OT[d, q]
        pending_tr = []

        def flush_tr(pool=None, tag="pp"):
            while pending_tr:
                qc, hp, onorm = pending_tr.pop(0)
                tp = (pool or ps_proj).tile([128, 512], f32, tag=tag,
                                            name=f"tp_{qc}_{hp}")
                tpb = tp[:, :].bitcast(bf16)
                for qb in range(4):
                    nc.tensor.matmul(tpb[:, qb * 128:(qb + 1) * 128],
                                     lhsT=onorm[:, qb, :], rhs=id_sb,
                                     start=True, stop=True,
                                     is_transpose=True)
                nc.vector.tensor_copy(out=OT_tiles[qc][:, hp, :],
                                      in_=tpb[:, 0:512])

        def attn_hp(qc, hp, fills=None, fills_mid=None, fills_post=None,
                    tr_at=2):
            qsl = slice(qc * CW, (qc + 1) * CW)
            po = [ps_o.tile([128, 512], f32, tag="po",
                            name=f"po_{qc}_{hp}_{hh}") for hh in range(2)]

            def qk_exp(kt):
                pss = ps_s.tile([128, 1024], f32, tag="pss",
                                name=f"pss_{qc}_{hp}_{kt}")
                for hh in range(2):
                    pb = hh * 64
                    mm(pss[:, hh * 512:(hh + 1) * 512],
                       KT_sb[pb:pb + 64, hp, kt * 128:(kt + 1) * 128],
                       QT_sb[pb:pb + 64, hp, qsl], True, True)
                e = exp_pool.tile([128, 1024], bf16, tag="ex",
                                  name=f"ex_{qc}_{hp}_{kt}")
                nc.scalar.activation(out=e, in_=pss, func=AF.Exp,
                                     scale=SCALE)
                return e

            def av(kt, e):
                # PSUM zero regions are bank-granular: start only on the
                # first matmul into each po bank (marks the whole bank
                # pending-zero; first touch of each byte overwrites), stop
                # only on the last.
                for hh in range(2):
                    h = 2 * hp + hh
                    for qb in range(4):
                        mm(po[hh][:, qb * 65:qb * 65 + 65],
                           e[:, hh * 512 + qb * 128:hh * 512 + (qb + 1) * 128],
                           Vp_sb[:, kt, h * 65:(h + 1) * 65],
                           kt == 0 and qb == 0, kt == 15 and qb == 3)

            # AV runs TWO k-tiles behind QK/exp so it never waits on the
            # exp stream (ACT is at most 2 tiles behind PE via the pss
            # double buffer)
            e_hist = []
            for kt in range(KTN):
                if fills:
                    for th in fills.get(kt, []):
                        th()
                if kt == tr_at:
                    flush_tr()
                e = qk_exp(kt)
                if fills_mid:
                    for th in fills_mid.get(kt, []):
                        th()
                e_hist.append(e)
                if kt >= 2:
                    av(kt - 2, e_hist[kt - 2])
            if fills_mid:
                for th in fills_mid.get(KTN, []):
                    th()
            av(KTN - 2, e_hist[KTN - 2])
            av(KTN - 1, e_hist[KTN - 1])
            if fills_post:
                for th in fills_post:
                    th()

            # drain: reciprocal of denominators (col 64 of each 65-block),
            # normalize into SBUF staging [q, d] (bf16)
            onorm = on_pool.tile([128, 4, 128], bf16, tag="on",
                                 name=f"on_{qc}_{hp}")
            rec = rec_pool.tile([128, 8], f32, tag="rec",
                                name=f"rec_{qc}_{hp}")
            import concourse.mybir as _mb
            for hh in range(2):
                nc.vector.reciprocal(out=rec[:, hh * 4:hh * 4 + 4],
                                     in_=po[hh][:, 64:260:65])
                pv = po[hh][:, 0:260].rearrange(
                    "p (qb c) -> p qb c", c=65)[:, :, 0:DK]
                rv = rec[:, hh * 4:hh * 4 + 4][:, :, None].broadcast_to(
                    [128, 4, DK])
                nc.vector.tensor_tensor(
                    out=onorm[:, :, hh * 64:(hh + 1) * 64],
                    in0=pv, in1=rv, op=_mb.AluOpType.mult)
            pending_tr.append((qc, hp, onorm))

        # ---------------- schedule ----------------
        # prefix DMAs (sync queue order = arrival order on the DMA device):
        # Q-projection path first (it gates the first QK/exp), then V/K.
        nc.sync.dma_start(out=wk_sb[:, 0], in_=wk_r[:, 0])
        load_xr("k", 0, 0, 256)
        nc.sync.dma_start(out=wq_sb[:, 0], in_=wq_r[:, 0])
        load_xr("q", 0, 0, 256)
        load_xr("q", 0, 256, 512)
        load_xr("k", 0, 256, 512)
        nc.sync.dma_start(out=wv_sb[:, 0], in_=wv_r[:, 0])
        load_xr("v", 0, 0, 256)
        load_xr("v", 0, 256, 512)
        load_x("k", 1)
        nc.sync.dma_start(out=wq_sb[:, 1:4], in_=wq_r[:, 1:4])
        nc.sync.dma_start(out=wv_sb[:, 1], in_=wv_r[:, 1])

        warm(15)
        kq_proj_h("k", 0, 0, 0)
        warm(4)
        kq_proj_h("q", 0, 0, 0)
        kq_proj_h("q", 0, 0, 1)

        def th(fn, *a):
            return lambda: fn(*a)

        def wrest(w_sb, w_r):
            return lambda: nc.sync.dma_start(out=w_sb[:, 1:4],
                                             in_=w_r[:, 1:4])

        for qc in range(QC):
            OT_tiles[qc] = ot_pool.tile([128, 4, 512], bf16, tag="OT",
                                        name=f"OT_{qc}")
            for hp in range(HPN):
                fills, mid, post = {}, {}, []
                tr_at = 2
                if qc == 0:
                    # K proj for THIS head pair's dj slice runs just-in-time
                    # (tc0 was emitted at the previous pair's boundary); one
                    # iter of slack so the DVE bias-add isn't on the QK path
                    for kt in (4, 8, 12):
                        if hp > 0 and kt == 4:
                            mid.setdefault(1, []).append(
                                th(k_proj, hp, 1))
                            continue
                        fills.setdefault(kt if hp == 0 else kt - 1,
                                         []).append(
                            th(k_proj, hp, kt // 4))
                    if hp < 3:
                        post.append(th(k_proj, hp + 1, 0))
                    if hp == 0:
                        # V head-half A (heads 0-3) just-in-time; Q proj
                        # covers the exp-pipeline warm-up bubbles; x/w
                        # prefetches spread down the DMA queue
                        for k in range(16):
                            mid.setdefault(min(k + 2, 16), []).append(
                                th(v_proj, k, 0))
                        fills.setdefault(1, []).append(
                            th(kq_proj_h, "k", 0, 0, 1))
                        fills.setdefault(5, []).append(th(q_proj, 0, 1))
                        fills.setdefault(11, []).append(th(q_proj, 0, 2))
                        fills.setdefault(13, []).append(th(q_proj, 0, 3))
                        fills.setdefault(0, []).insert(0, th(load_x, "v", 1))
                        fills.setdefault(2, []).insert(0, th(load_x, "k", 2))
                        fills.setdefault(4, []).insert(0, th(load_x, "v", 2))
                        fills.setdefault(6, []).insert(0, th(load_x, "k", 3))
                        fills.setdefault(8, []).insert(0, th(load_x, "v", 3))
                        fills.setdefault(10, []).insert(0, wrest(wk_sb, wk_r))
                    elif hp == 1:
                        # V head-half B (heads 4-7), first 10 tiles
                        for k in range(10):
                            mid.setdefault(k + 1, []).append(
                                th(v_proj, k, 1))
                        fills.setdefault(2, []).insert(
                            0, th(load_x, "q", 1))
                    elif hp == 2:
                        for k in range(10, 16):
                            mid.setdefault(k - 9, []).append(
                                th(v_proj, k, 1))
                        fills.setdefault(2, []).insert(
                            0, lambda: nc.sync.dma_start(out=wo_sb,
                                                         in_=wo_r))
                    else:
                        qa, qb = q_proj_split(1, 0)
                        fills.setdefault(5, []).append(qa)
                        fills.setdefault(9, []).append(qb)
                        fills.setdefault(2, []).insert(
                            0, th(load_x, "q", 2))
                else:
                    last = qc == 3
                    # each window JIT-fills ONE Q chain: for the next head
                    # pair's dj slice (or the next chunk's dj0 at hp3)
                    tqc, tdj = (qc, hp + 1) if hp < 3 else (qc + 1, 0)
                    if hp == 0:
                        tr_at = 4   # give qc-1/hp3's drain time to land
                        fills.setdefault(6, []).append(
                            th(o_proj, qc - 1, 0))
                        qa, qb = q_proj_split(tqc, tdj)
                        mid.setdefault(1, []).append(qa)
                        fills.setdefault(10, []).append(qb)
                        post.append(th(o_proj, qc - 1, 1))
                    else:
                        mid.setdefault(1, []).append(
                            th(o_proj, qc - 1, 2 * hp))
                        if tqc <= 3:
                            qa, qb = q_proj_split(tqc, tdj)
                            fills.setdefault(5, []).append(qa)
                            fills.setdefault(9, []).append(qb)
                        if not (last and hp == 3):
                            post.append(th(o_proj, qc - 1, 2 * hp + 1))
                    if qc == 1 and hp == 3:
                        fills.setdefault(13, []).insert(
                            0, th(load_x, "q", 3))
                    if last and hp == 3:
                        # tail overlap: leftover o_proj plus partial (kj<3)
                        # o_proj(3,*) chains run while the final drain chain
                        # (DVE normalize -> PE transpose -> OT copy) lands
                        tail_ab = [o_proj_split(3, 0),
                                   o_proj_split(3, 1, pool=ps_s,
                                                on_act=True),
                                   o_proj_split(3, 2, pool=ps_s,
                                                on_act=True),
                                   o_proj_split(3, 3)]
                        mid.setdefault(16, []).extend(
                            [th(o_proj, 2, 7), tail_ab[0][0]])
                        post.extend([tail_ab[1][0], tail_ab[2][0],
                                     tail_ab[3][0]])
                attn_hp(qc, hp, fills, mid, post, tr_at)
        # the final transpose staging borrows the (now idle) AV-accumulator
        # banks so a fourth o_proj chain can stay open across the flush
        flush_tr(pool=ps_o, tag="po")
        for a, b in tail_ab:
            b()
        for dj in range(4, 8):
            for t in o_proj_split(3, dj, pool=ps_s if dj % 2 else None,
                                  on_act=bool(dj % 2)):
                t()


def _prep_inputs(query, key, value, Wq, bq, Wk, bk, Wv, bv, Wo, bo):
    import ml_dtypes
    bf = ml_dtypes.bfloat16

    query = np.asarray(query, np.float32)
    key = np.asarray(key, np.float32)
    value = np.asarray(value, np.float32)
    Wq = np.asarray(Wq, np.float32)
    Wk = np.asarray(Wk, np.float32)
    Wv = np.asarray(Wv, np.float32)
    Wo = np.asarray(Wo, np.float32)
    bq = np.asarray(bq, np.float32)
    bk = np.asarray(bk, np.float32)
    bv = np.asarray(bv, np.float32)
    bo = np.asarray(bo, np.float32)

    ident = np.ascontiguousarray(np.eye(128, dtype=np.float32).astype(bf))

    xT = {}
    for b in range(B):
        xT[b] = (np.ascontiguousarray(query[b].T.astype(bf)),
                 np.ascontiguousarray(key[b].T.astype(bf)),
                 np.ascontiguousarray(value[b].T.astype(bf)))

    def blk4(wT):
        # [1024, 512] -> [128 p, 4 dj, 8 kj, 128 c]
        return np.ascontiguousarray(
            wT.reshape(8, 128, 4, 128).transpose(1, 2, 0, 3).astype(bf))

    def blk2(wT):
        # [1024, 512] -> [128 p, 2 half, 8 kj, 256 c]
        return np.ascontiguousarray(
            wT.reshape(8, 128, 2, 256).transpose(1, 2, 0, 3).astype(bf))

    grp = {}
    for g in range(2):
        gs = slice(DG * g, DG * (g + 1))
        bo_eff = Wo[:, gs] @ bv[gs]
        if g == 0:
            bo_eff = bo_eff + bo
        grp[g] = {
            "wq_in": blk4(Wq.T[:, gs]),
            "wk_in": blk4(Wk.T[:, gs]),
            "wv_in": blk2(Wv.T[:, gs]),
            "woT": np.ascontiguousarray(Wo.T[gs, :].astype(bf)),
            "bq_in": np.ascontiguousarray(bq[gs].reshape(4, 128).T),
            "bk_in": np.ascontiguousarray(bk[gs].reshape(4, 128).T),
            "bo_in": np.ascontiguousarray(bo_eff.reshape(8, 128).T),
            "id_in": ident,
        }

    in_maps = []
    for c in range(N_CORES):
        b, g = c // 2, c % 2
        m = {"xqT": xT[b][0], "xkT": xT[b][1], "xvT": xT[b][2]}
        m.update(grp[g])
        in_maps.append(m)
    return in_maps


def kernel(query, key, value, Wq, bq, Wk, bk, Wv, bv, Wo, bo):
    from concourse.bass_utils import run_bass_kernel_spmd

    if "nc" not in _CACHE:
        _CACHE["nc"] = _build_program()
    nc = _CACHE["nc"]

    in_maps = _prep_inputs(query, key, value, Wq, bq, Wk, bk, Wv, bv, Wo, bo)
    res = run_bass_kernel_spmd(nc, in_maps, list(range(N_CORES)))
    out = np.empty((B, S, D), np.float32)
    for b in range(B):
        y = (np.asarray(res.results[2 * b]["yT"], np.float32)
             + np.asarray(res.results[2 * b + 1]["yT"], np.float32))
        out[b] = y.T
    return out


# revision 112
# speedup vs baseline: 1.0045x; 1.0045x over previous
"""Multi-head attention (B=4, S=2048, D=1024, H=16) on 8 trn2 NeuronCores.

Sharding: batch x head-group (tensor parallel over heads). Core c handles
batch c//2 and heads (c%2)*8 .. (c%2)*8+7: it projects Q/K/V only for its
512 head dims (columns of Wq/Wk/Wv), runs attention for its 8 heads over
the full 2048-token sequence, and computes the PARTIAL output projection
y_g = O_g @ Wo[:, g-slice]^T (+ bias folded into group 0). The host adds
the two partials per batch during unshard - the row-sharded-Wo all-reduce
of standard tensor parallelism. No K/V projection duplication and no
cross-core traffic on device.

Device-side layout notes:
 - Activations stay transposed ([feature, token]); scores are computed as
   S^T[k, q] = K_h Q_h^T with head pairs stacked in partition halves
   (2 matmuls per 128-k tile, N=512 each), one ScalarE exp per [128,1024]
   PSUM tile.
 - AV is restructured for minimal PE row-streaming: the exp tile e[k, q]
   is the STATIONARY operand (lhsT, [128k x 128q] slices) and the moving
   operand is [V_h | ones] ([128k x 65]) so each matmul streams only 65
   rows -> out[q, 64+1] accumulates O[q, dk] AND the softmax denominator
   (col 64) over the 16 k-tiles. Per-partition reciprocal + tensor_scalar
   normalization (no cross-partition replicate needed), then a PE
   transpose (vs a DMA'd 128x128 identity) restores O^T[d, q] for the
   output projection.
 - AV runs TWO k-tiles behind the QK/exp stream (the scores double buffer
   caps ACT's lag at 2 tiles), so AV matmuls never stall on the exp that
   was just issued - this keeps every steady-state window at the ScalarE
   floor of 16 exp instructions.
 - Projections (K, Q, V, O) are emitted just-in-time inside the attention
   kt-loops to fill PE during ACT-bound stretches, split into half-chains
   (shared PSUM accumulation) so no fill lump exceeds what the 2-deep exp
   pipeline can absorb; V is projected in two head-halves (heads 0-3
   feed pairs 0-1, heads 4-7 feed pairs 2-3) to spread its cost. A few
   warm-up matmuls on a zeroed scratch tile hold the PE p-state ramp
   while the first DMAs land.
 - V bias is folded into the (partial) output-projection bias host-side;
   partial outputs are written back as bf16 (summed in f32 on the host).
 - PSUM budget: scores 2x[128,1024] + AV accumulators 2x[128,512] +
   projection/transpose 2x[128,512] = 8 banks exactly.
"""

import numpy as np

B, S, D, H = 4, 2048, 1024, 16
DK = D // H          # 64
HL = H // 2          # 8 local heads per core
DG = HL * DK         # 512 local head dims
CW = 512             # token chunk width
QC = S // CW         # 4 query chunks
KTN = S // 128       # 16 k tiles
HPN = HL // 2        # 4 local head pairs
SCALE = 1.0 / np.sqrt(DK)
N_CORES = 8

_CACHE = {}


def _build_program(reps=1):
    import concourse.bass as bass
    import concourse.mybir as mybir
    from concourse import bacc
    from concourse.tile import TileContext

    f32 = mybir.dt.float32
    bf16 = mybir.dt.bfloat16
    AF = mybir.ActivationFunctionType

    nc = bacc.Bacc("TRN2", target_bir_lowering=False)

    xqT = nc.declare_dram_parameter("xqT", [D, S], bf16, isOutput=False)
    xkT = nc.declare_dram_parameter("xkT", [D, S], bf16, isOutput=False)
    xvT = nc.declare_dram_parameter("xvT", [D, S], bf16, isOutput=False)
    # weights pre-blocked host-side to partition-major so sliced DMA loads
    # move 2KB+ contiguous runs (256B runs pay a 2x descriptor latency)
    wq_in = nc.declare_dram_parameter("wq_in", [128, 4, 8, 128], bf16,
                                      isOutput=False)
    wk_in = nc.declare_dram_parameter("wk_in", [128, 4, 8, 128], bf16,
                                      isOutput=False)
    wv_in = nc.declare_dram_parameter("wv_in", [128, 2, 8, 256], bf16,
                                      isOutput=False)
    woT = nc.declare_dram_parameter("woT", [DG, D], bf16, isOutput=False)
    bq_in = nc.declare_dram_parameter("bq_in", [128, 4], f32, isOutput=False)
    bk_in = nc.declare_dram_parameter("bk_in", [128, 4], f32, isOutput=False)
    bo_in = nc.declare_dram_parameter("bo_in", [128, 8], f32, isOutput=False)
    id_in = nc.declare_dram_parameter("id_in", [128, 128], bf16,
                                      isOutput=False)
    yT = nc.declare_dram_parameter("yT", [D, S], bf16, isOutput=True)

    xq_r = xqT[:].rearrange("(a p) t -> p a t", p=128)
    xk_r = xkT[:].rearrange("(a p) t -> p a t", p=128)
    xv_r = xvT[:].rearrange("(a p) t -> p a t", p=128)
    wq_r = wq_in[:]
    wk_r = wk_in[:]
    wv_r = wv_in[:]
    wo_r = woT[:].rearrange("(a p) d -> p a d", p=128)

    with TileContext(nc) as tc:
        for _rep in range(reps):
            _emit_body(nc, tc, bass, f32, bf16, AF,
                       xq_r, xk_r, xv_r, wq_r, wk_r, wv_r, wo_r,
                       bq_in, bk_in, bo_in, id_in, yT)
    nc.compile()
    return nc


def _emit_body(nc, tc, bass, f32, bf16, AF,
               xq_r, xk_r, xv_r, wq_r, wk_r, wv_r, wo_r,
               bq_in, bk_in, bo_in, id_in, yT):
    def mm(out, lhsT, rhs, start, stop):
        nc.tensor.matmul(out, lhsT=lhsT, rhs=rhs, start=start, stop=stop)

    with (
        tc.tile_pool(name="const", bufs=1) as const_pool,
        tc.tile_pool(name="kt_res", bufs=1) as kt_pool,
        tc.tile_pool(name="qt_res", bufs=1) as qt_pool,
        tc.tile_pool(name="vp_res", bufs=1) as vp_pool,
        tc.tile_pool(name="w_res", bufs=1) as w_pool,
        tc.tile_pool(name="xk_p", bufs=4) as xk_pool,
        tc.tile_pool(name="xv_p", bufs=4) as xv_pool,
        tc.tile_pool(name="xq_p", bufs=2) as xq_pool,
        tc.tile_pool(name="exp_p", bufs=6) as exp_pool,
        tc.tile_pool(name="on_p", bufs=3) as on_pool,
        tc.tile_pool(name="rec_p", bufs=3) as rec_pool,
        tc.tile_pool(name="ot_res", bufs=3) as ot_pool,
        tc.tile_pool(name="y_p", bufs=6) as y_pool,
        tc.tile_pool(name="ps_proj", bufs=2, space="PSUM") as ps_proj,
        tc.tile_pool(name="ps_s", bufs=2, space="PSUM") as ps_s,
        tc.tile_pool(name="ps_o", bufs=2, space="PSUM") as ps_o,
    ):
        bq_sb = const_pool.tile([128, 4], f32, tag="bq")
        bk_sb = const_pool.tile([128, 4], f32, tag="bk")
        bo_sb = const_pool.tile([128, 8], f32, tag="bo")
        id_sb = const_pool.tile([128, 128], bf16, tag="ident")
        nc.gpsimd.dma_start(out=bk_sb, in_=bk_in[:])
        nc.gpsimd.dma_start(out=bq_sb, in_=bq_in[:])
        nc.gpsimd.dma_start(out=bo_sb, in_=bo_in[:])
        nc.gpsimd.dma_start(out=id_sb, in_=id_in[:])

        KT_sb = kt_pool.tile([128, 4, S], bf16, tag="KT")   # [p, dj, t]
        QT_sb = qt_pool.tile([128, 4, S], bf16, tag="QT")   # [p, dj, t]
        Vp_sb = vp_pool.tile([128, KTN, HL * (DK + 1)], bf16, tag="Vp")
        Vp4 = Vp_sb.rearrange("p i (hh c) -> p i hh c", c=DK + 1)

        # scratch for PE p-state warm-up (zeros; results unused) —
        # memset first so the first warm matmul starts ASAP
        wsc = const_pool.tile([128, 384], bf16, tag="wsc")
        nc.vector.memset(wsc, 0.0)
        nc.vector.memset(Vp4[:, :, :, DK], 1.0)

        warm_ctr = [0]

        def warm(n):
            for _ in range(n):
                ps = ps_s.tile([128, 1024], f32, tag="pss",
                               name=f"warm_{warm_ctr[0]}")
                warm_ctr[0] += 1
                mm(ps[:, 0:256], wsc[:, 0:128], wsc[:, 128:384], True, True)

        wk_sb = w_pool.tile([128, 4, 8, 128], bf16, tag="wk")
        wq_sb = w_pool.tile([128, 4, 8, 128], bf16, tag="wq")
        wv_sb = w_pool.tile([128, 2, 8, 256], bf16, tag="wv")
        wo_sb = w_pool.tile([128, 4, D], bf16, tag="wo")

        xk_t, xv_t, xq_t = {}, {}, {}

        def load_xr(kind, tc_i, lo, hi):
            pool, cache, src = {
                "k": (xk_pool, xk_t, xk_r),
                "v": (xv_pool, xv_t, xv_r),
                "q": (xq_pool, xq_t, xq_r),
            }[kind]
            if tc_i not in cache:
                cache[tc_i] = pool.tile([128, 8, CW], bf16, tag=f"x{kind}",
                                        name=f"x{kind}_{tc_i}")
            base = tc_i * CW
            nc.sync.dma_start(out=cache[tc_i][:, :, lo:hi],
                              in_=src[:, :, base + lo:base + hi])

        def load_x(kind, tc_i):
            load_xr(kind, tc_i, 0, CW)

        def k_proj(dj, tci):
            ps = ps_proj.tile([128, 512], f32, tag="pp",
                              name=f"pk_{dj}_{tci}")
            for kj in range(8):
                mm(ps, wk_sb[:, dj, kj, :],
                   xk_t[tci][:, kj, :], kj == 0, kj == 7)
            nc.vector.tensor_scalar_add(
                out=KT_sb[:, dj, tci * CW:(tci + 1) * CW],
                in0=ps, scalar1=bk_sb[:, dj:dj + 1])

        def kq_proj_h(which, dj, tci, half):
            # half-width (256-token) chain: lets the prefix start on the
            # first half-chunk DMA instead of waiting for a full chunk
            w_sb, x_t, b_sb, out_sb = (
                (wk_sb, xk_t, bk_sb, KT_sb) if which == "k"
                else (wq_sb, xq_t, bq_sb, QT_sb))
            ps = ps_proj.tile([128, 512], f32, tag="pp",
                              name=f"p{which}h_{dj}_{tci}_{half}")
            lo = half * 256
            for kj in range(8):
                mm(ps[:, 0:256], w_sb[:, dj, kj, :],
                   x_t[tci][:, kj, lo:lo + 256], kj == 0, kj == 7)
            nc.vector.tensor_scalar_add(
                out=out_sb[:, dj, tci * CW + lo:tci * CW + lo + 256],
                in0=ps[:, 0:256], scalar1=b_sb[:, dj:dj + 1])

        def q_proj(qc, dj):
            for t in q_proj_split(qc, dj):
                t()

        def q_proj_split(qc, dj):
            # two half-chains sharing one PSUM accumulation, so the fill
            # can be spread across the window without a >1.4us PE lump
            box = {}

            def a():
                box["ps"] = ps_proj.tile([128, 512], f32, tag="pp",
                                         name=f"pq_{qc}_{dj}")
                for kj in range(4):
                    mm(box["ps"], wq_sb[:, dj, kj, :],
                       xq_t[qc][:, kj, :], kj == 0, False)

            def b():
                ps = box["ps"]
                for kj in range(4, 8):
                    mm(ps, wq_sb[:, dj, kj, :],
                       xq_t[qc][:, kj, :], False, kj == 7)
                nc.vector.tensor_scalar_add(
                    out=QT_sb[:, dj, qc * CW:(qc + 1) * CW],
                    in0=ps, scalar1=bq_sb[:, dj:dj + 1])

            return a, b

        def v_proj(ti, half):
            # half 0: heads 0-3 (used by hp0/hp1), half 1: heads 4-7
            tci, ts = ti // 4, ti % 4
            ps = ps_proj.tile([128, 512], f32, tag="pp",
                              name=f"pv_{ti}_{half}")
            for kj in range(8):
                mm(ps[:, 0:256], xv_t[tci][:, kj, ts * 128:(ts + 1) * 128],
                   wv_sb[:, half, kj, :], kj == 0, kj == 7)
            nc.vector.tensor_copy(
                out=Vp4[:, ti, half * 4:(half + 1) * 4, 0:DK],
                in_=ps[:, 0:256].rearrange("p (hh c) -> p hh c", c=DK))

        OT_tiles = {}

        def o_proj_split(qc, dj, pool=None, on_act=False):
            box = {}

            def a():
                p = pool or ps_proj
                t = p.tile([128, 512] if p is ps_proj else [128, 1024],
                           f32, tag="pp" if p is ps_proj else "pss",
                           name=f"py_{qc}_{dj}")
                box["ps"] = t[:, 0:512]
                for kj in range(3):
                    mm(box["ps"], wo_sb[:, kj, dj * 128:(dj + 1) * 128],
                       OT_tiles[qc][:, kj, :], kj == 0, False)

            def b():
                ps_y = box["ps"]
                mm(ps_y, wo_sb[:, 3, dj * 128:(dj + 1) * 128],
                   OT_tiles[qc][:, 3, :], False, True)
                yt = y_pool.tile([128, 512], bf16, tag="yt",
                                 name=f"yt_{qc}_{dj}")
                if on_act:
                    # tail-only: ACT is idle after the last exp; keep the
                    # serial DVE queue off the finish-line critical path
                    nc.scalar.activation(out=yt, in_=ps_y, func=AF.Identity,
                                         bias=bo_sb[:, dj:dj + 1], scale=1.0)
                else:
                    nc.vector.tensor_scalar_add(
                        out=yt, in0=ps_y, scalar1=bo_sb[:, dj:dj + 1])
                nc.sync.dma_start(
                    out=yT[dj * 128:(dj + 1) * 128, qc * CW:(qc + 1) * CW],
                    in_=yt)

            return a, b

        def o_proj(qc, dj, pool=None):
            for t in o_proj_split(qc, dj, pool):
                t()

        # deferred per-head-pair transposes: Onorm[q, d] -> OT[d, q]
        pending_tr = []

        def flush_tr(pool=None, tag="pp"):
            while pending_tr:
                qc, hp, onorm = pending_tr.pop(0)
                tp = (pool or ps_proj).tile([128, 512], f32, tag=tag,
                                            name=f"tp_{qc}_{hp}")
                tpb = tp[:, :].bitcast(bf16)
                for qb in range(4):
                    nc.tensor.matmul(tpb[:, qb * 128:(qb + 1) * 128],
                                     lhsT=onorm[:, qb, :], rhs=id_sb,
                                     start=True, stop=True,
                                     is_transpose=True)
                nc.vector.tensor_copy(out=OT_tiles[qc][:, hp, :],
                                      in_=tpb[:, 0:512])

        def attn_hp(qc, hp, fills=None, fills_mid=None, fills_post=None,
                    tr_at=2):
            qsl = slice(qc * CW, (qc + 1) * CW)
            po = [ps_o.tile([128, 512], f32, tag="po",
                            name=f"po_{qc}_{hp}_{hh}") for hh in range(2)]

            def qk_exp(kt):
                pss = ps_s.tile([128, 1024], f32, tag="pss",
                                name=f"pss_{qc}_{hp}_{kt}")
                for hh in range(2):
                    pb = hh * 64
                    mm(pss[:, hh * 512:(hh + 1) * 512],
                       KT_sb[pb:pb + 64, hp, kt * 128:(kt + 1) * 128],
                       QT_sb[pb:pb + 64, hp, qsl], True, True)
                e = exp_pool.tile([128, 1024], bf16, tag="ex",
                                  name=f"ex_{qc}_{hp}_{kt}")
                nc.scalar.activation(out=e, in_=pss, func=AF.Exp,
                                     scale=SCALE)
                return e

            def av(kt, e):
                # PSUM zero regions are bank-granular: start only on the
                # first matmul into each po bank (marks the whole bank
                # pending-zero; first touch of each byte overwrites), stop
                # only on the last.
                for hh in range(2):
                    h = 2 * hp + hh
                    for qb in range(4):
                        mm(po[hh][:, qb * 65:qb * 65 + 65],
                           e[:, hh * 512 + qb * 128:hh * 512 + (qb + 1) * 128],
                           Vp_sb[:, kt, h * 65:(h + 1) * 65],
                           kt == 0 and qb == 0, kt == 15 and qb == 3)

            # AV runs TWO k-tiles behind QK/exp so it never waits on the
            # exp stream (ACT is at most 2 tiles behind PE via the pss
            # double buffer)
            e_hist = []
            for kt in range(KTN):
                if fills:
                    for th in fills.get(kt, []):
                        th()
                if kt == tr_at:
                    flush_tr()
                e = qk_exp(kt)
                if fills_mid:
                    for th in fills_mid.get(kt, []):
                        th()
                e_hist.append(e)
                if kt >= 2:
                    av(kt - 2, e_hist[kt - 2])
            if fills_mid:
                for th in fills_mid.get(KTN, []):
                    th()
            av(KTN - 2, e_hist[KTN - 2])
            av(KTN - 1, e_hist[KTN - 1])
            if fills_post:
                for th in fills_post:
                    th()

            # drain: reciprocal of denominators (col 64 of each 65-block),
            # normalize into SBUF staging [q, d] (bf16)
            onorm = on_pool.tile([128, 4, 128], bf16, tag="on",
                                 name=f"on_{qc}_{hp}")
            rec = rec_pool.tile([128, 8], f32, tag="rec",
                                name=f"rec_{qc}_{hp}")
            import concourse.mybir as _mb
            for hh in range(2):
                nc.vector.reciprocal(out=rec[:, hh * 4:hh * 4 + 4],
                                     in_=po[hh][:, 64:260:65])
                pv = po[hh][:, 0:260].rearrange(
                    "p (qb c) -> p qb c", c=65)[:, :, 0:DK]
                rv = rec[:, hh * 4:hh * 4 + 4][:, :, None].broadcast_to(
                    [128, 4, DK])
                nc.vector.tensor_tensor(
                    out=onorm[:, :, hh * 64:(hh + 1) * 64],
                    in0=pv, in1=rv, op=_mb.AluOpType.mult)
            pending_tr.append((qc, hp, onorm))

        # ---------------- schedule ----------------
        # prefix DMAs (sync queue order = arrival order on the DMA device):
        # Q-projection path first (it gates the first QK/exp), then V/K.
        nc.sync.dma_start(out=wk_sb[:, 0], in_=wk_r[:, 0])
        load_xr("k", 0, 0, 256)
        nc.sync.dma_start(out=wq_sb[:, 0], in_=wq_r[:, 0])
        load_xr("q", 0, 0, 256)
        load_xr("q", 0, 256, 512)
        load_xr("k", 0, 256, 512)
        nc.sync.dma_start(out=wv_sb[:, 0], in_=wv_r[:, 0])
        load_xr("v", 0, 0, 256)
        load_xr("v", 0, 256, 512)
        load_x("k", 1)
        nc.sync.dma_start(out=wq_sb[:, 1:4], in_=wq_r[:, 1:4])
        nc.sync.dma_start(out=wv_sb[:, 1], in_=wv_r[:, 1])

        warm(15)
        kq_proj_h("k", 0, 0, 0)
        warm(4)
        kq_proj_h("q", 0, 0, 0)
        kq_proj_h("q", 0, 0, 1)

        def th(fn, *a):
            return lambda: fn(*a)

        def wrest(w_sb, w_r):
            return lambda: nc.sync.dma_start(out=w_sb[:, 1:4],
                                             in_=w_r[:, 1:4])

        for qc in range(QC):
            OT_tiles[qc] = ot_pool.tile([128, 4, 512], bf16, tag="OT",
                                        name=f"OT_{qc}")
            for hp in range(HPN):
                fills, mid, post = {}, {}, []
                tr_at = 2
                if qc == 0:
                    # K proj for THIS head pair's dj slice runs just-in-time
                    # (tc0 was emitted at the previous pair's boundary); one
                    # iter of slack so the DVE bias-add isn't on the QK path
                    for kt in (4, 8, 12):
                        if hp > 0 and kt == 4:
                            mid.setdefault(1, []).append(
                                th(k_proj, hp, 1))
                            continue
                        fills.setdefault(kt if hp == 0 else kt - 1,
                                         []).append(
                            th(k_proj, hp, kt // 4))
                    if hp < 3:
                        post.append(th(k_proj, hp + 1, 0))
                    if hp == 0:
                        # V head-half A (heads 0-3) just-in-time; Q proj
                        # covers the exp-pipeline warm-up bubbles; x/w
                        # prefetches spread down the DMA queue
                        for k in range(16):
                            slot = k + 2 if k < 2 else min(k + 1, 16)
                            mid.setdefault(slot, []).append(
                                th(v_proj, k, 0))
                        fills.setdefault(1, []).append(
                            th(kq_proj_h, "k", 0, 0, 1))
                        fills.setdefault(5, []).append(th(q_proj, 0, 1))
                        fills.setdefault(11, []).append(th(q_proj, 0, 2))
                        fills.setdefault(13, []).append(th(q_proj, 0, 3))
                        fills.setdefault(0, []).insert(0, th(load_x, "v", 1))
                        fills.setdefault(2, []).insert(0, th(load_x, "k", 2))
                        fills.setdefault(4, []).insert(0, th(load_x, "v", 2))
                        fills.setdefault(6, []).insert(0, th(load_x, "k", 3))
                        fills.setdefault(8, []).insert(0, th(load_x, "v", 3))
                        fills.setdefault(10, []).insert(0, wrest(wk_sb, wk_r))
                    elif hp == 1:
                        # V head-half B (heads 4-7), first 10 tiles
                        for k in range(10):
                            mid.setdefault(k + 1, []).append(
                                th(v_proj, k, 1))
                        fills.setdefault(2, []).insert(
                            0, th(load_x, "q", 1))
                    elif hp == 2:
                        for k in range(10, 16):
                            mid.setdefault(k - 9, []).append(
                                th(v_proj, k, 1))
                        fills.setdefault(2, []).insert(
                            0, lambda: nc.sync.dma_start(out=wo_sb,
                                                         in_=wo_r))
                    else:
                        qa, qb = q_proj_split(1, 0)
                        fills.setdefault(5, []).append(qa)
                        fills.setdefault(9, []).append(qb)
                        fills.setdefault(2, []).insert(
                            0, th(load_x, "q", 2))
                else:
                    last = qc == 3
                    # each window JIT-fills ONE Q chain: for the next head
                    # pair's dj slice (or the next chunk's dj0 at hp3)
                    tqc, tdj = (qc, hp + 1) if hp < 3 else (qc + 1, 0)
                    if hp == 0:
                        tr_at = 4   # give qc-1/hp3's drain time to land
                        fills.setdefault(6, []).append(
                            th(o_proj, qc - 1, 0))
                        qa, qb = q_proj_split(tqc, tdj)
                        mid.setdefault(1, []).append(qa)
                        fills.setdefault(10, []).append(qb)
                        post.append(th(o_proj, qc - 1, 1))
                    else:
                        mid.setdefault(1, []).append(
                            th(o_proj, qc - 1, 2 * hp))
                        if tqc <= 3:
                            qa, qb = q_proj_split(tqc, tdj)
                            fills.setdefault(5, []).append(qa)
                            fills.setdefault(9, []).append(qb)
                        if not (last and hp == 3):
                            post.append(th(o_proj, qc - 1, 2 * hp + 1))
                    if qc == 1 and hp == 3:
                        fills.setdefault(13, []).insert(
                            0, th(load_x, "q", 3))
                    if last and hp == 3:
                        # tail overlap: leftover o_proj plus partial (kj<3)
                        # o_proj(3,*) chains run while the final drain chain
                        # (DVE normalize -> PE transpose -> OT copy) lands
                        tail_ab = [o_proj_split(3, 0),
                                   o_proj_split(3, 1, pool=ps_s,
                                                on_act=True),
                                   o_proj_split(3, 2, pool=ps_s,
                                                on_act=True),
                                   o_proj_split(3, 3)]
                        mid.setdefault(16, []).extend(
                            [th(o_proj, 2, 7), tail_ab[0][0]])
                        post.extend([tail_ab[1][0], tail_ab[2][0],
                                     tail_ab[3][0]])
                attn_hp(qc, hp, fills, mid, post, tr_at)
        # the final transpose staging borrows the (now idle) AV-accumulator
        # banks so a fourth o_proj chain can stay open across the flush
        flush_tr(pool=ps_o, tag="po")
        for a, b in tail_ab:
            b()
        for dj in range(4, 8):
            for t in o_proj_split(3, dj, pool=ps_s if dj % 2 else None,
                                  on_act=bool(dj % 2)):
                t()


def _prep_inputs(query, key, value, Wq, bq, Wk, bk, Wv, bv, Wo, bo):
    import ml_dtypes
    bf = ml_dtypes.bfloat16

    query = np.asarray(query, np.float32)
    key = np.asarray(key, np.float32)
    value = np.asarray(value, np.float32)
    Wq = np.asarray(Wq, np.float32)
    Wk = np.asarray(Wk, np.float32)
    Wv = np.asarray(Wv, np.float32)
    Wo = np.asarray(Wo, np.float32)
    bq = np.asarray(bq, np.float32)
    bk = np.asarray(bk, np.float32)
    bv = np.asarray(bv, np.float32)
    bo = np.asarray(bo, np.float32)

    ident = np.ascontiguousarray(np.eye(128, dtype=np.float32).astype(bf))

    xT = {}
    for b in range(B):
        xT[b] = (np.ascontiguousarray(query[b].T.astype(bf)),
                 np.ascontiguousarray(key[b].T.astype(bf)),
                 np.ascontiguousarray(value[b].T.astype(bf)))

    def blk4(wT):
        # [1024, 512] -> [128 p, 4 dj, 8 kj, 128 c]
        return np.ascontiguousarray(
            wT.reshape(8, 128, 4, 128).transpose(1, 2, 0, 3).astype(bf))

    def blk2(wT):
        # [1024, 512] -> [128 p, 2 half, 8 kj, 256 c]
        return np.ascontiguousarray(
            wT.reshape(8, 128, 2, 256).transpose(1, 2, 0, 3).astype(bf))

    grp = {}
    for g in range(2):
        gs = slice(DG * g, DG * (g + 1))
        bo_eff = Wo[:, gs] @ bv[gs]
        if g == 0:
            bo_eff = bo_eff + bo
        grp[g] = {
            "wq_in": blk4(Wq.T[:, gs]),
            "wk_in": blk4(Wk.T[:, gs]),
            "wv_in": blk2(Wv.T[:, gs]),
            "woT": np.ascontiguousarray(Wo.T[gs, :].astype(bf)),
            "bq_in": np.ascontiguousarray(bq[gs].reshape(4, 128).T),
            "bk_in": np.ascontiguousarray(bk[gs].reshape(4, 128).T),
            "bo_in": np.ascontiguousarray(bo_eff.reshape(8, 128).T),
            "id_in": ident,
        }

    in_maps = []
    for c in range(N_CORES):
        b, g = c // 2, c % 2
        m = {"xqT": xT[b][0], "xkT": xT[b][1], "xvT": xT[b][2]}
        m.update(grp[g])
        in_maps.append(m)
    return in_maps


def kernel(query, key, value, Wq, bq, Wk, bk, Wv, bv, Wo, bo):
    from concourse.bass_utils import run_bass_kernel_spmd

    if "nc" not in _CACHE:
        _CACHE["nc"] = _build_program()
    nc = _CACHE["nc"]

    in_maps = _prep_inputs(query, key, value, Wq, bq, Wk, bk, Wv, bv, Wo, bo)
    res = run_bass_kernel_spmd(nc, in_maps, list(range(N_CORES)))
    out = np.empty((B, S, D), np.float32)
    for b in range(B):
        y = (np.asarray(res.results[2 * b]["yT"], np.float32)
             + np.asarray(res.results[2 * b + 1]["yT"], np.float32))
        out[b] = y.T
    return out
